# revision 3
# baseline (speedup 1.0000x reference)
"""CharRNN (LSTM + per-step linear/softmax) Trainium2 Bass kernel, 8 NeuronCores.

Strategy:
  - Recurrence (sequential over T=2048) is replicated on all 8 cores in bf16
    with weights SBUF-resident: gates = [h; x_t] @ Wcat.T accumulated in PSUM
    over 10 K-tiles, per 512-wide gate chunk (W rows pre-permuted so chunk n
    holds [i|f|g|o] for hidden slice n).
  - Per-step per-chunk elementwise on ACT/DVE; h re-transposed to [128,64]
    tiles via PE transpose for the next step's lhsT.
  - Each core stores the full hT history to DRAM; output projection + softmax
    (parallel part) is sharded over time: core c computes y for its 256 steps.
"""

import numpy as np
import ml_dtypes

T_SEQ, B, X, H = 2048, 64, 256, 1024
NCOR = 8
G4 = 4 * H  # 4096
KT_H, KT_X = 8, 2  # k-tiles for h (1024/128) and x (256/128)
NCHUNK = 8  # gate chunks of 512
TW = T_SEQ // NCOR  # 256 timesteps per core for phase P
PCH = TW // 2  # 128 phase-P chunks of 2 timesteps (128 rows)

_COMPILED = None


def _build_program():
    import concourse.bass as bass
    import concourse.mybir as mybir
    import concourse.tile as tile
    from concourse import bacc

    f32 = mybir.dt.float32
    bf16 = mybir.dt.float16  # fp16: 8x smaller mantissa error than bf16, same PE rate
    i32 = mybir.dt.int32
    AF = mybir.ActivationFunctionType
    EP = mybir.EngineType
    ds = bass.ds

    nc = bacc.Bacc("TRN2", target_bir_lowering=False, debug=False,
                   num_devices=NCOR)

    # ---- I/O ----
    xd = nc.dram_tensor("x", [128, 2 * T_SEQ * B], bf16, kind="ExternalInput").ap()
    wd = nc.dram_tensor("w", [128, (KT_H + KT_X) * G4], bf16, kind="ExternalInput").ap()
    biasg = nc.dram_tensor("biasg", [B, G4], f32, kind="ExternalInput").ap()
    h0T = nc.dram_tensor("h0T", [128, 512], bf16, kind="ExternalInput").ap()
    c0d = nc.dram_tensor("c0", [B, H], f32, kind="ExternalInput").ap()
    identd = nc.dram_tensor("ident", [B, B], f32, kind="ExternalInput").ap()
    wod = nc.dram_tensor("wo", [128, KT_H * X], bf16, kind="ExternalInput").ap()
    biaso = nc.dram_tensor("biaso", [128, X], f32, kind="ExternalInput").ap()
    cidd = nc.dram_tensor("cid", [1, 1], i32, kind="ExternalInput").ap()

    yout = nc.dram_tensor("y", [TW * B, X], f32, kind="ExternalOutput").ap()
    hout = nc.dram_tensor("hT_fin", [B, H], f32, kind="ExternalOutput").ap()
    cout = nc.dram_tensor("cT_fin", [B, H], f32, kind="ExternalOutput").ap()

    hist = nc.dram_tensor("hist", [T_SEQ, 128, 512], bf16).ap()

    # ---- persistent SBUF state ----
    w_sb = nc.alloc_sbuf_tensor("w_sb", [128, (KT_H + KT_X) * G4], bf16).ap()
    bias_sb = nc.alloc_sbuf_tensor("bias_sb", [B, G4], f32).ap()
    hT_a = nc.alloc_sbuf_tensor("hT_a", [128, 512], bf16).ap()
    hT_b = nc.alloc_sbuf_tensor("hT_b", [128, 512], bf16).ap()
    c_sb = nc.alloc_sbuf_tensor("c_sb", [B, H], f32).ap()
    hf_sb = nc.alloc_sbuf_tensor("hf_sb", [B, H], f32).ap()
    id_sb = nc.alloc_sbuf_tensor("id_sb", [B, B], f32).ap()
    wo_sb = nc.alloc_sbuf_tensor("wo_sb", [128, KT_H * X], bf16).ap()
    bo_sb = nc.alloc_sbuf_tensor("bo_sb", [128, X], f32).ap()
    cid_sb = nc.alloc_sbuf_tensor("cid_sb", [128, 1], i32).ap()

    SS = 8  # steps per superstep
    NSS = T_SEQ // SS

    with tile.TileContext(nc) as tc:
        with (
            tc.tile_pool(name="xp", bufs=2) as xpool,
            tc.tile_pool(name="gp", bufs=3) as gpool,
            tc.tile_pool(name="ap", bufs=3) as apool,
            tc.tile_pool(name="pp", bufs=4, space="PSUM") as ppool,
            tc.tile_pool(name="tp", bufs=2, space="PSUM") as tpool,
            tc.tile_pool(name="p3", bufs=2, space="PSUM") as p3pool,
            tc.tile_pool(name="s3", bufs=3) as s3pool,
        ):
            # setup DMAs
            nc.sync.dma_start(w_sb[:, :], wd[:, :])
            nc.sync.dma_start(bias_sb[:, :], biasg[:, :])
            nc.sync.dma_start(hT_a[:, :], h0T[:, :])
            nc.sync.dma_start(c_sb[:, :], c0d[:, :])
            nc.sync.dma_start(id_sb[:, :], identd[:, :])
            nc.sync.dma_start(wo_sb[:, :], wod[:, :])
            nc.sync.dma_start(bo_sb[:, :], biaso[:, :])
            nc.sync.dma_start(cid_sb[0:1, :], cidd[:, :])
            cid = nc.values_load(cid_sb[0:1, 0:1], engines=[EP.SP],
                                 min_val=0, max_val=NCOR - 1,
                                 skip_runtime_bounds_check=True)

            with tc.For_i(0, NSS, 1,
                          hint_engines=(EP.PE, EP.DVE, EP.Activation, EP.SP)) as ss:
                xt = xpool.tile([128, SS * B * KT_X], bf16)
                for kx in range(KT_X):
                    nc.sync.dma_start(
                        xt[:, kx * SS * B:(kx + 1) * SS * B],
                        xd[:, ds(kx * T_SEQ * B + ss * SS * B, SS * B)])
                for j in range(SS):
                    src_h, dst_h = (hT_a, hT_b) if j % 2 == 0 else (hT_b, hT_a)
                    for n in range(NCHUNK):
                        ps = ppool.tile([B, 512], f32)
                        for k in range(KT_H):
                            nc.tensor.matmul(
                                ps[:, :], src_h[:, 64 * k:64 * k + 64],
                                w_sb[:, G4 * k + 512 * n:G4 * k + 512 * n + 512],
                                start=(k == 0), stop=False)
                        for kx in range(KT_X):
                            nc.tensor.matmul(
                                ps[:, :],
                                xt[:, kx * SS * B + j * B:kx * SS * B + (j + 1) * B],
                                w_sb[:, G4 * (KT_H + kx) + 512 * n:
                                     G4 * (KT_H + kx) + 512 * n + 512],
                                start=False, stop=(kx == KT_X - 1))
                        gt = gpool.tile([B, 512], f32)
                        nc.vector.tensor_add(gt[:, :], ps[:, :],
                                             bias_sb[:, 512 * n:512 * n + 512])
                        i_s = apool.tile([B, 128], f32, tag="i_s")
                        f_s = apool.tile([B, 128], f32, tag="f_s")
                        g_s = apool.tile([B, 128], f32, tag="g_s")
                        o_s = apool.tile([B, 128], f32, tag="o_s")
                        nc.scalar.activation(i_s[:, :], gt[:, 0:128], AF.Sigmoid)
                        nc.scalar.activation(f_s[:, :], gt[:, 128:256], AF.Sigmoid)
                        nc.scalar.activation(g_s[:, :], gt[:, 256:384], AF.Tanh)
                        nc.scalar.activation(o_s[:, :], gt[:, 384:512], AF.Sigmoid)
                        fc = apool.tile([B, 128], f32, tag="fc")
                        ig = apool.tile([B, 128], f32, tag="ig")
                        cs = c_sb[:, 128 * n:128 * n + 128]
                        nc.vector.tensor_mul(fc[:, :], f_s[:, :], cs)
                        nc.vector.tensor_mul(ig[:, :], i_s[:, :], g_s[:, :])
                        nc.vector.tensor_add(cs, fc[:, :], ig[:, :])
                        th = apool.tile([B, 128], f32, tag="th")
                        nc.scalar.activation(th[:, :], cs, AF.Tanh)
                        hs = hf_sb[:, 128 * n:128 * n + 128]
                        nc.vector.tensor_mul(hs, o_s[:, :], th[:, :])
                        pt = tpool.tile([128, B], f32)
                        nc.tensor.transpose(pt[:, :], hs, id_sb[:, :])
                        nc.vector.tensor_copy(dst_h[:, 64 * n:64 * n + 64], pt[:, :])
                    nc.sync.dma_start(hist[ds(ss * SS + j, 1), :, :], dst_h[:, :])

            # final state out (batch-major, fp32)
            nc.sync.dma_start(hout[:, :], hf_sb[:, :])
            nc.sync.dma_start(cout[:, :], c_sb[:, :])

            # ---- phase P: per-core time slice, projection + softmax ----
            for ch in range(PCH):
                hin = s3pool.tile([128, 1024], bf16, tag="hin")
                # interleave the two timesteps so k-tile k is contiguous at
                # cols [128k, 128k+128) = [t0 batch | t1 batch]
                hin_k = hin[:, :].rearrange("p (k c) -> p k c", k=KT_H)
                nc.sync.dma_start(hin_k[:, :, 0:64],
                                  hist[ds(cid * TW + 2 * ch, 1), :, :])
                nc.sync.dma_start(hin_k[:, :, 64:128],
                                  hist[ds(cid * TW + 2 * ch + 1, 1), :, :])
                ps = p3pool.tile([128, X], f32)
                for k in range(KT_H):
                    nc.tensor.matmul(ps[:, :], hin[:, 128 * k:128 * k + 128],
                                     wo_sb[:, X * k:X * k + X],
                                     start=(k == 0), stop=(k == KT_H - 1))
                gl = s3pool.tile([128, X], f32, tag="gl")
                nc.vector.tensor_add(gl[:, :], ps[:, :], bo_sb[:, :])
                ex = s3pool.tile([128, X], f32, tag="ex")
                nc.scalar.activation(ex[:, :], gl[:, :], AF.Exp)
                sm = s3pool.tile([128, 1], f32, tag="sm")
                nc.vector.reduce_sum(sm[:, :], ex[:, :], axis=mybir.AxisListType.X)
                rc = s3pool.tile([128, 1], f32, tag="rc")
                nc.vector.reciprocal(rc[:, :], sm[:, :])
                yt = s3pool.tile([128, X], f32, tag="yt")
                nc.vector.tensor_scalar_mul(yt[:, :], ex[:, :], rc[:, 0:1])
                nc.sync.dma_start(yout[128 * ch:128 * ch + 128, :], yt[:, :])

    nc.compile()
    return nc


def _prep_inputs(input, h0, c0, W_ih, W_hh, b_ih, b_hh, W_out, b_out):
    bf = np.float16
    # gate-row permutation: chunk n = [i_n | f_n | g_n | o_n], 128 rows each
    perm = np.empty(G4, dtype=np.int64)
    for n in range(NCHUNK * 4 // 4):
        pass
    idx = 0
    perm_list = []
    for n in range(8):
        for g in range(4):
            perm_list.append(np.arange(g * H + 128 * n, g * H + 128 * n + 128))
    perm = np.concatenate(perm_list)

    W_cat = np.concatenate([W_hh, W_ih], axis=1)[perm]            # [4096,1280]
    WT = np.ascontiguousarray(W_cat.T)                            # [1280,4096]
    w_host = WT.reshape(KT_H + KT_X, 128, G4).transpose(1, 0, 2).reshape(128, -1)
    w_host = w_host.astype(bf)

    bias = (b_ih + b_hh)[perm].astype(np.float32)
    bias_host = np.ascontiguousarray(np.broadcast_to(bias[None, :], (B, G4)))

    xT = np.ascontiguousarray(input.reshape(T_SEQ * B, X).T)      # [256, T*B]
    x_host = xT.reshape(KT_X, 128, T_SEQ * B).transpose(1, 0, 2).reshape(128, -1)
    x_host = x_host.astype(bf)

    h0T = np.ascontiguousarray(h0.T)                              # [1024, 64]
    h0_host = h0T.reshape(KT_H, 128, B).transpose(1, 0, 2).reshape(128, -1).astype(bf)

    c0_host = c0.astype(np.float32)

    ident = np.eye(B, dtype=np.float32)

    WoT = np.ascontiguousarray(W_out.T)                           # [1024, 256]
    wo_host = WoT.reshape(KT_H, 128, X).transpose(1, 0, 2).reshape(128, -1).astype(bf)

    bo_host = np.ascontiguousarray(
        np.broadcast_to(b_out.astype(np.float32)[None, :], (128, X)))

    base = {
        "x": x_host, "w": w_host, "biasg": bias_host, "h0T": h0_host,
        "c0": c0_host, "ident": ident, "wo": wo_host, "biaso": bo_host,
    }
    in_maps = []
    for c in range(NCOR):
        m = dict(base)
        m["cid"] = np.full((1, 1), c, np.int32)
        in_maps.append(m)
    return in_maps


def kernel(input, h0, c0, W_ih, W_hh, b_ih, b_hh, W_out, b_out):
    global _COMPILED
    from concourse.bass_utils import run_bass_kernel_spmd

    input = np.asarray(input, dtype=np.float32)
    h0 = np.asarray(h0, dtype=np.float32)
    c0 = np.asarray(c0, dtype=np.float32)
    W_ih = np.asarray(W_ih, dtype=np.float32)
    W_hh = np.asarray(W_hh, dtype=np.float32)
    b_ih = np.asarray(b_ih, dtype=np.float32)
    b_hh = np.asarray(b_hh, dtype=np.float32)
    W_out = np.asarray(W_out, dtype=np.float32)
    b_out = np.asarray(b_out, dtype=np.float32)

    if _COMPILED is None:
        _COMPILED = _build_program()
    nc = _COMPILED

    in_maps = _prep_inputs(input, h0, c0, W_ih, W_hh, b_ih, b_hh, W_out, b_out)
    res = run_bass_kernel_spmd(nc, in_maps, list(range(NCOR)))

    y = np.empty((T_SEQ, B, X), dtype=np.float32)
    for c in range(NCOR):
        y[c * TW:(c + 1) * TW] = res.results[c]["y"].reshape(TW, B, X)
    h_T = res.results[0]["hT_fin"]
    c_T = res.results[0]["cT_fin"]
    return y, h_T, c_T


# revision 4
# speedup vs baseline: 1.1893x; 1.1893x over previous
"""CharRNN (LSTM + per-step linear/softmax) Trainium2 Bass kernel, 8 NeuronCores.

Strategy:
  - Recurrence (sequential over T=2048) is replicated on all 8 cores in bf16
    with weights SBUF-resident: gates = [h; x_t] @ Wcat.T accumulated in PSUM
    over 10 K-tiles, per 512-wide gate chunk (W rows pre-permuted so chunk n
    holds [i|f|g|o] for hidden slice n).
  - Per-step per-chunk elementwise on ACT/DVE; h re-transposed to [128,64]
    tiles via PE transpose for the next step's lhsT.
  - Each core stores the full hT history to DRAM; output projection + softmax
    (parallel part) is sharded over time: core c computes y for its 256 steps.
"""

import numpy as np
import ml_dtypes

T_SEQ, B, X, H = 2048, 64, 256, 1024
NCOR = 8
G4 = 4 * H  # 4096
KT_H, KT_X = 8, 2  # k-tiles for h (1024/128) and x (256/128)
NCHUNK = 8  # gate chunks of 512
TW = T_SEQ // NCOR  # 256 timesteps per core for phase P
PCH = TW // 2  # 128 phase-P chunks of 2 timesteps (128 rows)

_COMPILED = None


def _build_program():
    import concourse.bass as bass
    import concourse.mybir as mybir
    import concourse.tile as tile
    from concourse import bacc

    f32 = mybir.dt.float32
    bf16 = mybir.dt.float16  # fp16: 8x smaller mantissa error than bf16, same PE rate
    i32 = mybir.dt.int32
    AF = mybir.ActivationFunctionType
    EP = mybir.EngineType
    ds = bass.ds

    nc = bacc.Bacc("TRN2", target_bir_lowering=False, debug=False,
                   num_devices=NCOR)

    # ---- I/O ----
    xd = nc.dram_tensor("x", [128, 2 * T_SEQ * B], bf16, kind="ExternalInput").ap()
    wd = nc.dram_tensor("w", [128, (KT_H + KT_X) * G4], bf16, kind="ExternalInput").ap()
    biasg = nc.dram_tensor("biasg", [B, G4], f32, kind="ExternalInput").ap()
    h0T = nc.dram_tensor("h0T", [128, 512], bf16, kind="ExternalInput").ap()
    c0d = nc.dram_tensor("c0", [B, H], f32, kind="ExternalInput").ap()
    identd = nc.dram_tensor("ident", [B, B], f32, kind="ExternalInput").ap()
    wod = nc.dram_tensor("wo", [128, KT_H * X], bf16, kind="ExternalInput").ap()
    biaso = nc.dram_tensor("biaso", [128, X], f32, kind="ExternalInput").ap()
    cidd = nc.dram_tensor("cid", [1, 1], i32, kind="ExternalInput").ap()

    yout = nc.dram_tensor("y", [TW * B, X], f32, kind="ExternalOutput").ap()
    hout = nc.dram_tensor("hT_fin", [B, H], f32, kind="ExternalOutput").ap()
    cout = nc.dram_tensor("cT_fin", [B, H], f32, kind="ExternalOutput").ap()

    hist = nc.dram_tensor("hist", [T_SEQ, 128, 512], bf16).ap()

    # ---- persistent SBUF state ----
    w_sb = nc.alloc_sbuf_tensor("w_sb", [128, (KT_H + KT_X) * G4], bf16).ap()
    bias_sb = nc.alloc_sbuf_tensor("bias_sb", [B, G4], f32).ap()
    hT_a = nc.alloc_sbuf_tensor("hT_a", [128, 512], bf16).ap()
    hT_b = nc.alloc_sbuf_tensor("hT_b", [128, 512], bf16).ap()
    c_sb = nc.alloc_sbuf_tensor("c_sb", [B, H], f32).ap()
    hf_sb = nc.alloc_sbuf_tensor("hf_sb", [B, H], f32).ap()
    id_sb = nc.alloc_sbuf_tensor("id_sb", [B, B], f32).ap()
    wo_sb = nc.alloc_sbuf_tensor("wo_sb", [128, KT_H * X], bf16).ap()
    bo_sb = nc.alloc_sbuf_tensor("bo_sb", [128, X], f32).ap()
    cid_sb = nc.alloc_sbuf_tensor("cid_sb", [128, 1], i32).ap()

    SS = 8  # steps per superstep
    NSS = T_SEQ // SS

    with tile.TileContext(nc) as tc:
        with (
            tc.tile_pool(name="xp", bufs=2) as xpool,
            tc.tile_pool(name="gp", bufs=3) as gpool,
            tc.tile_pool(name="ap", bufs=3) as apool,
            tc.tile_pool(name="pp", bufs=4, space="PSUM") as ppool,
            tc.tile_pool(name="tp", bufs=2, space="PSUM") as tpool,
            tc.tile_pool(name="p3", bufs=2, space="PSUM") as p3pool,
            tc.tile_pool(name="s3", bufs=3) as s3pool,
        ):
            # setup DMAs
            nc.sync.dma_start(w_sb[:, :], wd[:, :])
            nc.sync.dma_start(bias_sb[:, :], biasg[:, :])
            nc.sync.dma_start(hT_a[:, :], h0T[:, :])
            nc.sync.dma_start(c_sb[:, :], c0d[:, :])
            nc.sync.dma_start(id_sb[:, :], identd[:, :])
            nc.sync.dma_start(wo_sb[:, :], wod[:, :])
            nc.sync.dma_start(bo_sb[:, :], biaso[:, :])
            nc.sync.dma_start(cid_sb[0:1, :], cidd[:, :])
            cid = nc.values_load(cid_sb[0:1, 0:1], engines=[EP.SP],
                                 min_val=0, max_val=NCOR - 1,
                                 skip_runtime_bounds_check=True)

            with tc.For_i(0, NSS, 1,
                          hint_engines=(EP.PE, EP.DVE, EP.Activation, EP.SP)) as ss:
                xt = xpool.tile([128, SS * B * KT_X], bf16)
                for kx in range(KT_X):
                    nc.sync.dma_start(
                        xt[:, kx * SS * B:(kx + 1) * SS * B],
                        xd[:, ds(kx * T_SEQ * B + ss * SS * B, SS * B)])
                for j in range(SS):
                    src_h, dst_h = (hT_a, hT_b) if j % 2 == 0 else (hT_b, hT_a)
                    for n in range(NCHUNK):
                        ps = ppool.tile([B, 512], f32)
                        for k in range(KT_H):
                            nc.tensor.matmul(
                                ps[:, :], src_h[:, 64 * k:64 * k + 64],
                                w_sb[:, G4 * k + 512 * n:G4 * k + 512 * n + 512],
                                start=(k == 0), stop=False)
                        for kx in range(KT_X):
                            nc.tensor.matmul(
                                ps[:, :],
                                xt[:, kx * SS * B + j * B:kx * SS * B + (j + 1) * B],
                                w_sb[:, G4 * (KT_H + kx) + 512 * n:
                                     G4 * (KT_H + kx) + 512 * n + 512],
                                start=False, stop=(kx == KT_X - 1))
                        gt = gpool.tile([B, 512], f32)
                        nc.vector.tensor_add(gt[:, :], ps[:, :],
                                             bias_sb[:, 512 * n:512 * n + 512])
                        i_s = apool.tile([B, 128], f32, tag="i_s")
                        f_s = apool.tile([B, 128], f32, tag="f_s")
                        g_s = apool.tile([B, 128], f32, tag="g_s")
                        o_s = apool.tile([B, 128], f32, tag="o_s")
                        nc.scalar.activation(i_s[:, :], gt[:, 0:128], AF.Sigmoid)
                        nc.scalar.activation(f_s[:, :], gt[:, 128:256], AF.Sigmoid)
                        nc.scalar.activation(g_s[:, :], gt[:, 256:384], AF.Tanh)
                        nc.scalar.activation(o_s[:, :], gt[:, 384:512], AF.Sigmoid)
                        fc = apool.tile([B, 128], f32, tag="fc")
                        ig = apool.tile([B, 128], f32, tag="ig")
                        cs = c_sb[:, 128 * n:128 * n + 128]
                        nc.vector.tensor_mul(fc[:, :], f_s[:, :], cs)
                        nc.vector.tensor_mul(ig[:, :], i_s[:, :], g_s[:, :])
                        nc.vector.tensor_add(cs, fc[:, :], ig[:, :])
                        th = apool.tile([B, 128], f32, tag="th")
                        nc.scalar.activation(th[:, :], cs, AF.Tanh)
                        hs = hf_sb[:, 128 * n:128 * n + 128]
                        nc.vector.tensor_mul(hs, o_s[:, :], th[:, :])
                        pt = tpool.tile([128, B], f32)
                        nc.tensor.transpose(pt[:, :], hs, id_sb[:, :])
                        nc.vector.tensor_copy(dst_h[:, 64 * n:64 * n + 64], pt[:, :])
                    nc.sync.dma_start(hist[ds(ss * SS + j, 1), :, :], dst_h[:, :])

            # final state out (batch-major, fp32)
            nc.sync.dma_start(hout[:, :], hf_sb[:, :])
            nc.sync.dma_start(cout[:, :], c_sb[:, :])

            # ---- phase P: per-core time slice, projection + softmax ----
            for ch in range(PCH):
                hin = s3pool.tile([128, 1024], bf16, tag="hin")
                # interleave the two timesteps so k-tile k is contiguous at
                # cols [128k, 128k+128) = [t0 batch | t1 batch]
                hin_k = hin[:, :].rearrange("p (k c) -> p k c", k=KT_H)
                nc.sync.dma_start(hin_k[:, :, 0:64],
                                  hist[ds(cid * TW + 2 * ch, 1), :, :])
                nc.sync.dma_start(hin_k[:, :, 64:128],
                                  hist[ds(cid * TW + 2 * ch + 1, 1), :, :])
                ps = p3pool.tile([128, X], f32)
                for k in range(KT_H):
                    nc.tensor.matmul(ps[:, :], hin[:, 128 * k:128 * k + 128],
                                     wo_sb[:, X * k:X * k + X],
                                     start=(k == 0), stop=(k == KT_H - 1))
                gl = s3pool.tile([128, X], f32, tag="gl")
                nc.vector.tensor_add(gl[:, :], ps[:, :], bo_sb[:, :])
                ex = s3pool.tile([128, X], f32, tag="ex")
                nc.scalar.activation(ex[:, :], gl[:, :], AF.Exp)
                sm = s3pool.tile([128, 1], f32, tag="sm")
                nc.vector.reduce_sum(sm[:, :], ex[:, :], axis=mybir.AxisListType.X)
                rc = s3pool.tile([128, 1], f32, tag="rc")
                nc.vector.reciprocal(rc[:, :], sm[:, :])
                yt = s3pool.tile([128, X], f32, tag="yt")
                nc.vector.tensor_scalar_mul(yt[:, :], ex[:, :], rc[:, 0:1])
                nc.sync.dma_start(yout[128 * ch:128 * ch + 128, :], yt[:, :])

    nc.compile()
    return nc


def _prep_inputs(input, h0, c0, W_ih, W_hh, b_ih, b_hh, W_out, b_out):
    bf = np.float16
    # gate-row permutation: chunk n = [i_n | f_n | g_n | o_n], 128 rows each
    perm = np.concatenate(
        [np.arange(g * H + 128 * n, g * H + 128 * n + 128)
         for n in range(NCHUNK) for g in range(4)])

    W_cat = np.concatenate([W_hh, W_ih], axis=1)[perm]            # [4096,1280]
    WT = np.ascontiguousarray(W_cat.T)                            # [1280,4096]
    w_host = WT.reshape(KT_H + KT_X, 128, G4).transpose(1, 0, 2).reshape(128, -1)
    w_host = w_host.astype(bf)

    bias = (b_ih + b_hh)[perm].astype(np.float32)
    bias_host = np.ascontiguousarray(np.broadcast_to(bias[None, :], (B, G4)))

    xT = np.ascontiguousarray(input.reshape(T_SEQ * B, X).T)      # [256, T*B]
    x_host = xT.reshape(KT_X, 128, T_SEQ * B).transpose(1, 0, 2).reshape(128, -1)
    x_host = x_host.astype(bf)

    h0T = np.ascontiguousarray(h0.T)                              # [1024, 64]
    h0_host = h0T.reshape(KT_H, 128, B).transpose(1, 0, 2).reshape(128, -1).astype(bf)

    c0_host = c0.astype(np.float32)

    ident = np.eye(B, dtype=np.float32)

    WoT = np.ascontiguousarray(W_out.T)                           # [1024, 256]
    wo_host = WoT.reshape(KT_H, 128, X).transpose(1, 0, 2).reshape(128, -1).astype(bf)

    bo_host = np.ascontiguousarray(
        np.broadcast_to(b_out.astype(np.float32)[None, :], (128, X)))

    base = {
        "x": x_host, "w": w_host, "biasg": bias_host, "h0T": h0_host,
        "c0": c0_host, "ident": ident, "wo": wo_host, "biaso": bo_host,
    }
    in_maps = []
    for c in range(NCOR):
        m = dict(base)
        m["cid"] = np.full((1, 1), c, np.int32)
        in_maps.append(m)
    return in_maps


def kernel(input, h0, c0, W_ih, W_hh, b_ih, b_hh, W_out, b_out):
    global _COMPILED
    from concourse.bass_utils import run_bass_kernel_spmd

    input = np.asarray(input, dtype=np.float32)
    h0 = np.asarray(h0, dtype=np.float32)
    c0 = np.asarray(c0, dtype=np.float32)
    W_ih = np.asarray(W_ih, dtype=np.float32)
    W_hh = np.asarray(W_hh, dtype=np.float32)
    b_ih = np.asarray(b_ih, dtype=np.float32)
    b_hh = np.asarray(b_hh, dtype=np.float32)
    W_out = np.asarray(W_out, dtype=np.float32)
    b_out = np.asarray(b_out, dtype=np.float32)

    if _COMPILED is None:
        _COMPILED = _build_program()
    nc = _COMPILED

    in_maps = _prep_inputs(input, h0, c0, W_ih, W_hh, b_ih, b_hh, W_out, b_out)
    res = run_bass_kernel_spmd(nc, in_maps, list(range(NCOR)))

    y = np.empty((T_SEQ, B, X), dtype=np.float32)
    for c in range(NCOR):
        y[c * TW:(c + 1) * TW] = res.results[c]["y"].reshape(TW, B, X)
    h_T = res.results[0]["hT_fin"]
    c_T = res.results[0]["cT_fin"]
    return y, h_T, c_T


# revision 5
# speedup vs baseline: 1.2052x; 1.0134x over previous
"""CharRNN (LSTM + per-step linear/softmax) Trainium2 Bass kernel, 8 NeuronCores.

Strategy:
  - Recurrence (sequential over T=2048) is replicated on all 8 cores in bf16
    with weights SBUF-resident: gates = [h; x_t] @ Wcat.T accumulated in PSUM
    over 10 K-tiles, per 512-wide gate chunk (W rows pre-permuted so chunk n
    holds [i|f|g|o] for hidden slice n).
  - Per-step per-chunk elementwise on ACT/DVE; h re-transposed to [128,64]
    tiles via PE transpose for the next step's lhsT.
  - Each core stores the full hT history to DRAM; output projection + softmax
    (parallel part) is sharded over time: core c computes y for its 256 steps.
"""

import numpy as np
import ml_dtypes

T_SEQ, B, X, H = 2048, 64, 256, 1024
NCOR = 8
G4 = 4 * H  # 4096
KT_H, KT_X = 8, 2  # k-tiles for h (1024/128) and x (256/128)
NCHUNK = 8  # gate chunks of 512
TW = T_SEQ // NCOR  # 256 timesteps per core for phase P
PCH = TW // 2  # 128 phase-P chunks of 2 timesteps (128 rows)

_COMPILED = None


def _build_program():
    import concourse.bass as bass
    import concourse.mybir as mybir
    import concourse.tile as tile
    from concourse import bacc

    f32 = mybir.dt.float32
    bf16 = mybir.dt.float16  # fp16: 8x smaller mantissa error than bf16, same PE rate
    i32 = mybir.dt.int32
    AF = mybir.ActivationFunctionType
    EP = mybir.EngineType
    ds = bass.ds

    nc = bacc.Bacc("TRN2", target_bir_lowering=False, debug=False,
                   num_devices=NCOR)

    # ---- I/O ----
    xd = nc.dram_tensor("x", [128, 2 * T_SEQ * B], bf16, kind="ExternalInput").ap()
    wd = nc.dram_tensor("w", [128, (KT_H + KT_X) * G4], bf16, kind="ExternalInput").ap()
    biasg = nc.dram_tensor("biasg", [B, G4], f32, kind="ExternalInput").ap()
    h0T = nc.dram_tensor("h0T", [128, 512], bf16, kind="ExternalInput").ap()
    c0d = nc.dram_tensor("c0", [B, H], f32, kind="ExternalInput").ap()
    identd = nc.dram_tensor("ident", [B, B], f32, kind="ExternalInput").ap()
    wod = nc.dram_tensor("wo", [128, KT_H * X], bf16, kind="ExternalInput").ap()
    biaso = nc.dram_tensor("biaso", [128, X], f32, kind="ExternalInput").ap()
    cidd = nc.dram_tensor("cid", [1, 1], i32, kind="ExternalInput").ap()

    yout = nc.dram_tensor("y", [TW * B, X], f32, kind="ExternalOutput").ap()
    hout = nc.dram_tensor("hT_fin", [B, H], f32, kind="ExternalOutput").ap()
    cout = nc.dram_tensor("cT_fin", [B, H], f32, kind="ExternalOutput").ap()

    hist = nc.dram_tensor("hist", [T_SEQ, 128, 512], bf16).ap()

    # ---- persistent SBUF state ----
    w_sb = nc.alloc_sbuf_tensor("w_sb", [128, (KT_H + KT_X) * G4], bf16).ap()
    bias_sb = nc.alloc_sbuf_tensor("bias_sb", [B, G4], f32).ap()
    hT_a = nc.alloc_sbuf_tensor("hT_a", [128, 512], bf16).ap()
    hT_b = nc.alloc_sbuf_tensor("hT_b", [128, 512], bf16).ap()
    c_sb = nc.alloc_sbuf_tensor("c_sb", [B, H], f32).ap()
    hf_sb = nc.alloc_sbuf_tensor("hf_sb", [B, H], f32).ap()
    id_sb = nc.alloc_sbuf_tensor("id_sb", [B, B], f32).ap()
    wo_sb = nc.alloc_sbuf_tensor("wo_sb", [128, KT_H * X], bf16).ap()
    bo_sb = nc.alloc_sbuf_tensor("bo_sb", [128, X], f32).ap()
    cid_sb = nc.alloc_sbuf_tensor("cid_sb", [128, 1], i32).ap()

    SS = 8  # steps per superstep
    NSS = T_SEQ // SS

    with tile.TileContext(nc) as tc:
        with (
            tc.tile_pool(name="xp", bufs=2) as xpool,
            tc.tile_pool(name="gp", bufs=3) as gpool,
            tc.tile_pool(name="ap", bufs=3) as apool,
            tc.tile_pool(name="pp", bufs=6, space="PSUM") as ppool,
            tc.tile_pool(name="p3", bufs=2, space="PSUM") as p3pool,
            tc.tile_pool(name="s3", bufs=3) as s3pool,
        ):
            # setup DMAs
            nc.sync.dma_start(w_sb[:, :], wd[:, :])
            nc.sync.dma_start(bias_sb[:, :], biasg[:, :])
            nc.sync.dma_start(hT_a[:, :], h0T[:, :])
            nc.sync.dma_start(c_sb[:, :], c0d[:, :])
            nc.sync.dma_start(id_sb[:, :], identd[:, :])
            nc.sync.dma_start(wo_sb[:, :], wod[:, :])
            nc.sync.dma_start(bo_sb[:, :], biaso[:, :])
            nc.sync.dma_start(cid_sb[0:1, :], cidd[:, :])
            cid = nc.values_load(cid_sb[0:1, 0:1], engines=[EP.SP],
                                 min_val=0, max_val=NCOR - 1,
                                 skip_runtime_bounds_check=True)

            with tc.For_i(0, NSS, 1,
                          hint_engines=(EP.PE, EP.DVE, EP.Activation, EP.SP)) as ss:
                xt = xpool.tile([128, SS * B * KT_X], bf16)
                for kx in range(KT_X):
                    nc.sync.dma_start(
                        xt[:, kx * SS * B:(kx + 1) * SS * B],
                        xd[:, ds(kx * T_SEQ * B + ss * SS * B, SS * B)])
                for j in range(SS):
                    src_h, dst_h = (hT_a, hT_b) if j % 2 == 0 else (hT_b, hT_a)
                    for n in range(NCHUNK):
                        ps = ppool.tile([B, 512], f32)
                        for k in range(KT_H):
                            nc.tensor.matmul(
                                ps[:, :], src_h[:, 64 * k:64 * k + 64],
                                w_sb[:, G4 * k + 512 * n:G4 * k + 512 * n + 512],
                                start=(k == 0), stop=False)
                        for kx in range(KT_X):
                            nc.tensor.matmul(
                                ps[:, :],
                                xt[:, kx * SS * B + j * B:kx * SS * B + (j + 1) * B],
                                w_sb[:, G4 * (KT_H + kx) + 512 * n:
                                     G4 * (KT_H + kx) + 512 * n + 512],
                                start=False, stop=(kx == KT_X - 1))
                        gt = gpool.tile([B, 512], f32)
                        nc.vector.tensor_add(gt[:, :], ps[:, :],
                                             bias_sb[:, 512 * n:512 * n + 512])
                        i_s = apool.tile([B, 128], f32, tag="i_s")
                        f_s = apool.tile([B, 128], f32, tag="f_s")
                        g_s = apool.tile([B, 128], f32, tag="g_s")
                        o_s = apool.tile([B, 128], f32, tag="o_s")
                        nc.scalar.activation(i_s[:, :], gt[:, 0:128], AF.Sigmoid)
                        nc.scalar.activation(f_s[:, :], gt[:, 128:256], AF.Sigmoid)
                        nc.scalar.activation(g_s[:, :], gt[:, 256:384], AF.Tanh)
                        nc.scalar.activation(o_s[:, :], gt[:, 384:512], AF.Sigmoid)
                        fc = apool.tile([B, 128], f32, tag="fc")
                        ig = apool.tile([B, 128], f32, tag="ig")
                        cs = c_sb[:, 128 * n:128 * n + 128]
                        nc.vector.tensor_mul(fc[:, :], f_s[:, :], cs)
                        nc.vector.tensor_mul(ig[:, :], i_s[:, :], g_s[:, :])
                        nc.vector.tensor_add(cs, fc[:, :], ig[:, :])
                        th = apool.tile([B, 128], f32, tag="th")
                        nc.scalar.activation(th[:, :], cs, AF.Tanh)
                        hs = hf_sb[:, 128 * n:128 * n + 128]
                        nc.vector.tensor_mul(hs, o_s[:, :], th[:, :])
                        hh = apool.tile([B, 128], bf16, tag="hh")
                        nc.vector.tensor_copy(hh[:, :], hs)
                        nc.scalar.dma_start_transpose(
                            out=dst_h[:, 64 * n:64 * n + 64], in_=hh[:, :])
                    nc.sync.dma_start(hist[ds(ss * SS + j, 1), :, :], dst_h[:, :])

            # final state out (batch-major, fp32)
            nc.sync.dma_start(hout[:, :], hf_sb[:, :])
            nc.sync.dma_start(cout[:, :], c_sb[:, :])

            # ---- phase P: per-core time slice, projection + softmax ----
            for ch in range(PCH):
                hin = s3pool.tile([128, 1024], bf16, tag="hin")
                # interleave the two timesteps so k-tile k is contiguous at
                # cols [128k, 128k+128) = [t0 batch | t1 batch]
                hin_k = hin[:, :].rearrange("p (k c) -> p k c", k=KT_H)
                nc.sync.dma_start(hin_k[:, :, 0:64],
                                  hist[ds(cid * TW + 2 * ch, 1), :, :])
                nc.sync.dma_start(hin_k[:, :, 64:128],
                                  hist[ds(cid * TW + 2 * ch + 1, 1), :, :])
                ps = p3pool.tile([128, X], f32)
                for k in range(KT_H):
                    nc.tensor.matmul(ps[:, :], hin[:, 128 * k:128 * k + 128],
                                     wo_sb[:, X * k:X * k + X],
                                     start=(k == 0), stop=(k == KT_H - 1))
                gl = s3pool.tile([128, X], f32, tag="gl")
                nc.vector.tensor_add(gl[:, :], ps[:, :], bo_sb[:, :])
                ex = s3pool.tile([128, X], f32, tag="ex")
                nc.scalar.activation(ex[:, :], gl[:, :], AF.Exp)
                sm = s3pool.tile([128, 1], f32, tag="sm")
                nc.vector.reduce_sum(sm[:, :], ex[:, :], axis=mybir.AxisListType.X)
                rc = s3pool.tile([128, 1], f32, tag="rc")
                nc.vector.reciprocal(rc[:, :], sm[:, :])
                yt = s3pool.tile([128, X], f32, tag="yt")
                nc.vector.tensor_scalar_mul(yt[:, :], ex[:, :], rc[:, 0:1])
                nc.sync.dma_start(yout[128 * ch:128 * ch + 128, :], yt[:, :])

    nc.compile()
    return nc


def _prep_inputs(input, h0, c0, W_ih, W_hh, b_ih, b_hh, W_out, b_out):
    bf = np.float16
    # gate-row permutation: chunk n = [i_n | f_n | g_n | o_n], 128 rows each
    perm = np.concatenate(
        [np.arange(g * H + 128 * n, g * H + 128 * n + 128)
         for n in range(NCHUNK) for g in range(4)])

    W_cat = np.concatenate([W_hh, W_ih], axis=1)[perm]            # [4096,1280]
    WT = np.ascontiguousarray(W_cat.T)                            # [1280,4096]
    w_host = WT.reshape(KT_H + KT_X, 128, G4).transpose(1, 0, 2).reshape(128, -1)
    w_host = w_host.astype(bf)

    bias = (b_ih + b_hh)[perm].astype(np.float32)
    bias_host = np.ascontiguousarray(np.broadcast_to(bias[None, :], (B, G4)))

    xT = np.ascontiguousarray(input.reshape(T_SEQ * B, X).T)      # [256, T*B]
    x_host = xT.reshape(KT_X, 128, T_SEQ * B).transpose(1, 0, 2).reshape(128, -1)
    x_host = x_host.astype(bf)

    h0T = np.ascontiguousarray(h0.T)                              # [1024, 64]
    h0_host = h0T.reshape(KT_H, 128, B).transpose(1, 0, 2).reshape(128, -1).astype(bf)

    c0_host = c0.astype(np.float32)

    ident = np.eye(B, dtype=np.float32)

    WoT = np.ascontiguousarray(W_out.T)                           # [1024, 256]
    wo_host = WoT.reshape(KT_H, 128, X).transpose(1, 0, 2).reshape(128, -1).astype(bf)

    bo_host = np.ascontiguousarray(
        np.broadcast_to(b_out.astype(np.float32)[None, :], (128, X)))

    base = {
        "x": x_host, "w": w_host, "biasg": bias_host, "h0T": h0_host,
        "c0": c0_host, "ident": ident, "wo": wo_host, "biaso": bo_host,
    }
    in_maps = []
    for c in range(NCOR):
        m = dict(base)
        m["cid"] = np.full((1, 1), c, np.int32)
        in_maps.append(m)
    return in_maps


def kernel(input, h0, c0, W_ih, W_hh, b_ih, b_hh, W_out, b_out):
    global _COMPILED
    from concourse.bass_utils import run_bass_kernel_spmd

    input = np.asarray(input, dtype=np.float32)
    h0 = np.asarray(h0, dtype=np.float32)
    c0 = np.asarray(c0, dtype=np.float32)
    W_ih = np.asarray(W_ih, dtype=np.float32)
    W_hh = np.asarray(W_hh, dtype=np.float32)
    b_ih = np.asarray(b_ih, dtype=np.float32)
    b_hh = np.asarray(b_hh, dtype=np.float32)
    W_out = np.asarray(W_out, dtype=np.float32)
    b_out = np.asarray(b_out, dtype=np.float32)

    if _COMPILED is None:
        _COMPILED = _build_program()
    nc = _COMPILED

    in_maps = _prep_inputs(input, h0, c0, W_ih, W_hh, b_ih, b_hh, W_out, b_out)
    res = run_bass_kernel_spmd(nc, in_maps, list(range(NCOR)))

    y = np.empty((T_SEQ, B, X), dtype=np.float32)
    for c in range(NCOR):
        y[c * TW:(c + 1) * TW] = res.results[c]["y"].reshape(TW, B, X)
    h_T = res.results[0]["hT_fin"]
    c_T = res.results[0]["cT_fin"]
    return y, h_T, c_T


# revision 7
# speedup vs baseline: 1.2210x; 1.0131x over previous
"""CharRNN (LSTM + per-step linear/softmax) Trainium2 Bass kernel, 8 NeuronCores.

Strategy:
  - Recurrence (sequential over T=2048) is replicated on all 8 cores in fp16
    with weights SBUF-resident: gates = [h; x_t] @ Wcat.T accumulated in PSUM
    over 10 K-tiles, per 512-wide gate chunk (W rows pre-permuted so chunk n
    holds [i|f|g|o] for hidden slice n).
  - Per-step per-chunk elementwise on ACT/DVE; h re-transposed to [128,64]
    lhsT tiles via 2-byte DMA-transpose on the ACT HWDGE queue (keeps the PE
    free for matmuls).
  - The x-projection (+bias) is precomputed per 8-step superstep at full
    M=128 PE efficiency and injected via the DVE gate add, so the per-step
    k-loop is 8 tiles instead of 10 (~10% less PE streaming).
  - Each core stores the full hT history to DRAM; output projection + softmax
    (parallel part) is sharded over time: core c computes y for its 256 steps.
  - fp16 (not bf16) everywhere: same PE rate, 8x lower mantissa error; max
    rel err vs fp32 reference ~3.3e-3 over the 2048-step recurrence.
"""

import numpy as np
import ml_dtypes

T_SEQ, B, X, H = 2048, 64, 256, 1024
NCOR = 8
G4 = 4 * H  # 4096
KT_H, KT_X = 8, 2  # k-tiles for h (1024/128) and x (256/128)
NCHUNK = 8  # gate chunks of 512
TW = T_SEQ // NCOR  # 256 timesteps per core for phase P
PCH = TW // 2  # 128 phase-P chunks of 2 timesteps (128 rows)

_COMPILED = None


def _build_program():
    import concourse.bass as bass
    import concourse.mybir as mybir
    import concourse.tile as tile
    from concourse import bacc

    f32 = mybir.dt.float32
    bf16 = mybir.dt.float16  # fp16: 8x smaller mantissa error than bf16, same PE rate
    i32 = mybir.dt.int32
    AF = mybir.ActivationFunctionType
    EP = mybir.EngineType
    ds = bass.ds

    nc = bacc.Bacc("TRN2", target_bir_lowering=False, debug=False,
                   num_devices=NCOR)

    # ---- I/O ----
    xd = nc.dram_tensor("x", [128, 2 * T_SEQ * B], bf16, kind="ExternalInput").ap()
    wd = nc.dram_tensor("w", [128, (KT_H + KT_X) * G4], bf16, kind="ExternalInput").ap()
    biasg = nc.dram_tensor("biasg", [128, G4], f32, kind="ExternalInput").ap()
    h0T = nc.dram_tensor("h0T", [128, 512], bf16, kind="ExternalInput").ap()
    c0d = nc.dram_tensor("c0", [B, H], f32, kind="ExternalInput").ap()
    identd = nc.dram_tensor("ident", [B, B], f32, kind="ExternalInput").ap()
    wod = nc.dram_tensor("wo", [128, KT_H * X], bf16, kind="ExternalInput").ap()
    biaso = nc.dram_tensor("biaso", [128, X], f32, kind="ExternalInput").ap()
    cidd = nc.dram_tensor("cid", [1, 1], i32, kind="ExternalInput").ap()

    yout = nc.dram_tensor("y", [TW * B, X], f32, kind="ExternalOutput").ap()
    hout = nc.dram_tensor("hT_fin", [B, H], f32, kind="ExternalOutput").ap()
    cout = nc.dram_tensor("cT_fin", [B, H], f32, kind="ExternalOutput").ap()

    hist = nc.dram_tensor("hist", [T_SEQ, 128, 512], bf16).ap()

    # ---- persistent SBUF state ----
    w_sb = nc.alloc_sbuf_tensor("w_sb", [128, (KT_H + KT_X) * G4], bf16).ap()
    bias_sb = nc.alloc_sbuf_tensor("bias_sb", [128, G4], f32).ap()
    xp_sb = nc.alloc_sbuf_tensor("xp_sb", [128, 4 * G4], bf16).ap()
    hT_a = nc.alloc_sbuf_tensor("hT_a", [128, 512], bf16).ap()
    hT_b = nc.alloc_sbuf_tensor("hT_b", [128, 512], bf16).ap()
    c_sb = nc.alloc_sbuf_tensor("c_sb", [B, H], f32).ap()
    hf_sb = nc.alloc_sbuf_tensor("hf_sb", [B, H], f32).ap()
    id_sb = nc.alloc_sbuf_tensor("id_sb", [B, B], f32).ap()
    wo_sb = nc.alloc_sbuf_tensor("wo_sb", [128, KT_H * X], bf16).ap()
    bo_sb = nc.alloc_sbuf_tensor("bo_sb", [128, X], f32).ap()
    cid_sb = nc.alloc_sbuf_tensor("cid_sb", [128, 1], i32).ap()

    SS = 8  # steps per superstep
    NSS = T_SEQ // SS

    with tile.TileContext(nc) as tc:
        with (
            tc.tile_pool(name="xp", bufs=2) as xpool,
            tc.tile_pool(name="gp", bufs=3) as gpool,
            tc.tile_pool(name="ap", bufs=3) as apool,
            tc.tile_pool(name="pp", bufs=4, space="PSUM") as ppool,
            tc.tile_pool(name="xq", bufs=2, space="PSUM") as xqpool,
            tc.tile_pool(name="p3", bufs=2, space="PSUM") as p3pool,
            tc.tile_pool(name="s3", bufs=3) as s3pool,
        ):
            # setup DMAs
            nc.sync.dma_start(w_sb[:, :], wd[:, :])
            nc.sync.dma_start(bias_sb[:, :], biasg[:, :])
            nc.sync.dma_start(hT_a[:, :], h0T[:, :])
            nc.sync.dma_start(c_sb[:, :], c0d[:, :])
            nc.sync.dma_start(id_sb[:, :], identd[:, :])
            nc.sync.dma_start(wo_sb[:, :], wod[:, :])
            nc.sync.dma_start(bo_sb[:, :], biaso[:, :])
            nc.sync.dma_start(cid_sb[0:1, :], cidd[:, :])
            cid = nc.values_load(cid_sb[0:1, 0:1], engines=[EP.SP],
                                 min_val=0, max_val=NCOR - 1,
                                 skip_runtime_bounds_check=True)

            with tc.For_i(0, NSS, 1,
                          hint_engines=(EP.PE, EP.DVE, EP.Activation, EP.SP)) as ss:
                xt = xpool.tile([128, SS * B * KT_X], bf16)
                for kx in range(KT_X):
                    nc.sync.dma_start(
                        xt[:, kx * SS * B:(kx + 1) * SS * B],
                        xd[:, ds(kx * T_SEQ * B + ss * SS * B, SS * B)])
                for m in range(4):
                    for n in range(NCHUNK):
                        xps = xqpool.tile([128, 512], f32, tag="xps")
                        for k in range(KT_X):
                            nc.tensor.matmul(
                                xps[:, :],
                                xt[:, k * SS * B + 128 * m:k * SS * B + 128 * m + 128],
                                w_sb[:, G4 * (KT_H + k) + 512 * n:
                                     G4 * (KT_H + k) + 512 * n + 512],
                                start=(k == 0), stop=(k == KT_X - 1))
                        nc.vector.tensor_add(
                            xp_sb[:, G4 * m + 512 * n:G4 * m + 512 * n + 512],
                            xps[:, :], bias_sb[:, 512 * n:512 * n + 512])
                for j in range(SS):
                    src_h, dst_h = (hT_a, hT_b) if j % 2 == 0 else (hT_b, hT_a)
                    for n in range(NCHUNK):
                        ps = ppool.tile([B, 512], f32)
                        for k in range(KT_H):
                            nc.tensor.matmul(
                                ps[:, :], src_h[:, 64 * k:64 * k + 64],
                                w_sb[:, G4 * k + 512 * n:G4 * k + 512 * n + 512],
                                start=(k == 0), stop=(k == KT_H - 1))
                        gt = gpool.tile([B, 512], f32)
                        xoff = G4 * (j // 2) + 512 * n
                        nc.vector.tensor_add(
                            gt[:, :], ps[:, :],
                            xp_sb[64 * (j % 2):64 * (j % 2) + 64,
                                  xoff:xoff + 512])
                        i_s = apool.tile([B, 128], f32, tag="i_s")
                        f_s = apool.tile([B, 128], f32, tag="f_s")
                        g_s = apool.tile([B, 128], f32, tag="g_s")
                        o_s = apool.tile([B, 128], f32, tag="o_s")
                        nc.scalar.activation(i_s[:, :], gt[:, 0:128], AF.Sigmoid)
                        nc.scalar.activation(f_s[:, :], gt[:, 128:256], AF.Sigmoid)
                        nc.scalar.activation(g_s[:, :], gt[:, 256:384], AF.Tanh)
                        nc.scalar.activation(o_s[:, :], gt[:, 384:512], AF.Sigmoid)
                        fc = apool.tile([B, 128], f32, tag="fc")
                        ig = apool.tile([B, 128], f32, tag="ig")
                        cs = c_sb[:, 128 * n:128 * n + 128]
                        nc.vector.tensor_mul(fc[:, :], f_s[:, :], cs)
                        nc.vector.tensor_mul(ig[:, :], i_s[:, :], g_s[:, :])
                        nc.vector.tensor_add(cs, fc[:, :], ig[:, :])
                        th = apool.tile([B, 128], f32, tag="th")
                        nc.scalar.activation(th[:, :], cs, AF.Tanh)
                        hs = hf_sb[:, 128 * n:128 * n + 128]
                        nc.vector.tensor_mul(hs, o_s[:, :], th[:, :])
                        hh = apool.tile([B, 128], bf16, tag="hh")
                        nc.vector.tensor_copy(hh[:, :], hs)
                        nc.scalar.dma_start_transpose(
                            out=dst_h[:, 64 * n:64 * n + 64], in_=hh[:, :])
                    nc.sync.dma_start(hist[ds(ss * SS + j, 1), :, :], dst_h[:, :])

            # final state out (batch-major, fp32)
            nc.sync.dma_start(hout[:, :], hf_sb[:, :])
            nc.sync.dma_start(cout[:, :], c_sb[:, :])

            # ---- phase P: per-core time slice, projection + softmax ----
            for ch in range(PCH):
                hin = s3pool.tile([128, 1024], bf16, tag="hin")
                # interleave the two timesteps so k-tile k is contiguous at
                # cols [128k, 128k+128) = [t0 batch | t1 batch]
                hin_k = hin[:, :].rearrange("p (k c) -> p k c", k=KT_H)
                nc.sync.dma_start(hin_k[:, :, 0:64],
                                  hist[ds(cid * TW + 2 * ch, 1), :, :])
                nc.sync.dma_start(hin_k[:, :, 64:128],
                                  hist[ds(cid * TW + 2 * ch + 1, 1), :, :])
                ps = p3pool.tile([128, X], f32)
                for k in range(KT_H):
                    nc.tensor.matmul(ps[:, :], hin[:, 128 * k:128 * k + 128],
                                     wo_sb[:, X * k:X * k + X],
                                     start=(k == 0), stop=(k == KT_H - 1))
                gl = s3pool.tile([128, X], f32, tag="gl")
                nc.vector.tensor_add(gl[:, :], ps[:, :], bo_sb[:, :])
                ex = s3pool.tile([128, X], f32, tag="ex")
                nc.scalar.activation(ex[:, :], gl[:, :], AF.Exp)
                sm = s3pool.tile([128, 1], f32, tag="sm")
                nc.vector.reduce_sum(sm[:, :], ex[:, :], axis=mybir.AxisListType.X)
                rc = s3pool.tile([128, 1], f32, tag="rc")
                nc.vector.reciprocal(rc[:, :], sm[:, :])
                yt = s3pool.tile([128, X], f32, tag="yt")
                nc.vector.tensor_scalar_mul(yt[:, :], ex[:, :], rc[:, 0:1])
                nc.sync.dma_start(yout[128 * ch:128 * ch + 128, :], yt[:, :])

    nc.compile()
    return nc


def _prep_inputs(input, h0, c0, W_ih, W_hh, b_ih, b_hh, W_out, b_out):
    bf = np.float16
    # gate-row permutation: chunk n = [i_n | f_n | g_n | o_n], 128 rows each
    perm = np.concatenate(
        [np.arange(g * H + 128 * n, g * H + 128 * n + 128)
         for n in range(NCHUNK) for g in range(4)])

    W_cat = np.concatenate([W_hh, W_ih], axis=1)[perm]            # [4096,1280]
    WT = np.ascontiguousarray(W_cat.T)                            # [1280,4096]
    w_host = WT.reshape(KT_H + KT_X, 128, G4).transpose(1, 0, 2).reshape(128, -1)
    w_host = w_host.astype(bf)

    bias = (b_ih + b_hh)[perm].astype(np.float32)
    bias_host = np.ascontiguousarray(np.broadcast_to(bias[None, :], (128, G4)))

    xT = np.ascontiguousarray(input.reshape(T_SEQ * B, X).T)      # [256, T*B]
    x_host = xT.reshape(KT_X, 128, T_SEQ * B).transpose(1, 0, 2).reshape(128, -1)
    x_host = x_host.astype(bf)

    h0T = np.ascontiguousarray(h0.T)                              # [1024, 64]
    h0_host = h0T.reshape(KT_H, 128, B).transpose(1, 0, 2).reshape(128, -1).astype(bf)

    c0_host = c0.astype(np.float32)

    ident = np.eye(B, dtype=np.float32)

    WoT = np.ascontiguousarray(W_out.T)                           # [1024, 256]
    wo_host = WoT.reshape(KT_H, 128, X).transpose(1, 0, 2).reshape(128, -1).astype(bf)

    bo_host = np.ascontiguousarray(
        np.broadcast_to(b_out.astype(np.float32)[None, :], (128, X)))

    base = {
        "x": x_host, "w": w_host, "biasg": bias_host, "h0T": h0_host,
        "c0": c0_host, "ident": ident, "wo": wo_host, "biaso": bo_host,
    }
    in_maps = []
    for c in range(NCOR):
        m = dict(base)
        m["cid"] = np.full((1, 1), c, np.int32)
        in_maps.append(m)
    return in_maps


def kernel(input, h0, c0, W_ih, W_hh, b_ih, b_hh, W_out, b_out):
    global _COMPILED
    from concourse.bass_utils import run_bass_kernel_spmd

    input = np.asarray(input, dtype=np.float32)
    h0 = np.asarray(h0, dtype=np.float32)
    c0 = np.asarray(c0, dtype=np.float32)
    W_ih = np.asarray(W_ih, dtype=np.float32)
    W_hh = np.asarray(W_hh, dtype=np.float32)
    b_ih = np.asarray(b_ih, dtype=np.float32)
    b_hh = np.asarray(b_hh, dtype=np.float32)
    W_out = np.asarray(W_out, dtype=np.float32)
    b_out = np.asarray(b_out, dtype=np.float32)

    if _COMPILED is None:
        _COMPILED = _build_program()
    nc = _COMPILED

    in_maps = _prep_inputs(input, h0, c0, W_ih, W_hh, b_ih, b_hh, W_out, b_out)
    res = run_bass_kernel_spmd(nc, in_maps, list(range(NCOR)))

    y = np.empty((T_SEQ, B, X), dtype=np.float32)
    for c in range(NCOR):
        y[c * TW:(c + 1) * TW] = res.results[c]["y"].reshape(TW, B, X)
    h_T = res.results[0]["hT_fin"]
    c_T = res.results[0]["cT_fin"]
    return y, h_T, c_T
